# revision 37
# baseline (speedup 1.0000x reference)
"""AdaMoE layer on 8 Trainium2 NeuronCores — expert-parallel Bass/Tile kernel.

Strategy: each core k owns expert k. The host routes tokens with the same
bf16 gating math as the reference and builds ONE GLOBAL gathered token
stream per core (only the tokens this core's expert selects, ~65% of 4096),
padded to a capacity computed from the actual data (max count over cores,
rounded up to 128). Routing weights are folded into per-(sg, si) one-hot
permutation blocks (pm), so no gating runs on device. The device runs the
dense FFN in bf16 (fp32 PSUM accumulation) over the gathered stream in
groups of 384 tokens; each 128-token output tile is scattered back to dense
rows via banded permutation matmuls (band pairs computed from the data,
unioned across cores so the SPMD graph is identical on every core, which
the collectives require); dense 128-row tiles are combined across cores by
piece-wise ReduceScatter (512-row pieces, tapered to 256-row pieces at the
end so only one small collective trails the last matmul); the host
reassembles the shards.
"""

import numpy as np
import ml_dtypes

import concourse.bass as bass
import concourse.bacc as bacc
import concourse.mybir as mybir
import concourse.tile as tile
from concourse.tile_rust import add_dep_helper
from concourse.bass_utils import run_bass_kernel_spmd

BF16 = ml_dtypes.bfloat16

B, S, D, FF, E = 2, 2048, 1024, 4096, 8
T = B * S
NCORES = 8
MAX_THRESHOLD = 0.125

P = 128            # SBUF partitions
SUB = 128          # tokens per PE output subtile
KD = D // P        # 8 contraction chunks over D
KF = FF // P       # 32 contraction chunks over FF
NHALF = D // 512   # FFN2 output split (PSUM bank = 512 fp32)
W1PARTS = 8        # W1 DMA split (chained; early f-chunks land earliest)
W2PARTS = 4        # W2 DMA split
GRP = 384          # gathered tokens per FFN1 group (ht SBUF tile)
ROT = 0            # dense-row rotation of the gathered stream
NSI = T // SUB     # dense output subtiles (32)
# RS piece sizes in dense subtiles, in completion order (sum must be NSI).
# Tapered at the end so only small collectives are tail-exposed.
PIECE_SIZES = (4, 4, 4, 4, 4, 4, 2, 1, 1, 2, 2)
NTAILBF = 4        # trailing pieces whose ReduceScatter runs in bf16
                   # (small collectives are latency-bound; bf16 shaves ~3us
                   # each, and only these sit on the serialized tail chain)

dt = mybir.dt
Act = mybir.ActivationFunctionType
GELU_FUNC = Act.Gelu_apprx_tanh


def _build(cap, bands, pieces, n_cores=NCORES):
    """Build the SPMD graph (identical on every core).

    cap: gathered-stream length (multiple of SUB).
    bands: tuple over dense si of tuple-of-sg (ascending) covering it.
    pieces: tuple of (si_lo, n_si) in emission order.
    """
    nsg = cap // SUB
    comp = [max(b) for b in bands]           # completing sg per dense si
    # pm pair index in emission order: si's grouped by completion sg
    si_by_comp = [[] for _ in range(nsg)]
    for si in range(NSI):
        si_by_comp[comp[si]].append(si)
    pair_idx = {}
    for sg in range(nsg):
        for si in si_by_comp[sg]:
            for b in bands[si]:
                pair_idx[(si, b)] = len(pair_idx)
    n_pairs = len(pair_idx)
    # osb liveness window (in sg) for the banded un-gather
    osb_live = max(3, max(comp[si] - min(bands[si]) for si in range(NSI)) + 1)
    # groups of the gathered stream
    groups = []
    g0 = 0
    while g0 < cap:
        ts = min(GRP, cap - g0)
        groups.append((g0, ts))
        g0 += ts
    # piece bookkeeping: which piece each si belongs to + completion sg of piece
    si_piece = {}
    for pi, (lo, n) in enumerate(pieces):
        for si in range(lo, lo + n):
            si_piece[si] = pi
    piece_remaining = [n for (lo, n) in pieces]

    nc = bacc.Bacc(
        "TRN2",
        target_bir_lowering=False,
        debug=False,
        enable_asserts=True,
        num_devices=n_cores,
    )

    xT = nc.dram_tensor("xT", [D, cap], dt.bfloat16, kind="ExternalInput")
    pm_d = nc.dram_tensor("pm", [n_pairs, SUB, SUB], dt.bfloat16,
                          kind="ExternalInput")
    w1 = nc.dram_tensor("w1", [D, FF], dt.bfloat16, kind="ExternalInput")
    w2 = nc.dram_tensor("w2", [FF, D], dt.bfloat16, kind="ExternalInput")
    b1t = nc.dram_tensor("b1t", [FF], dt.float32, kind="ExternalInput")
    b2t = nc.dram_tensor("b2t", [D], dt.bfloat16, kind="ExternalInput")
    n_bf = sum(n for (_, n) in pieces[len(pieces) - NTAILBF :])
    n_f32 = NSI - n_bf
    out_ext = nc.dram_tensor(
        "out", [n_f32 * SUB // n_cores, D], dt.float32, kind="ExternalOutput"
    )
    outb_ext = nc.dram_tensor(
        "outb", [n_bf * SUB // n_cores, D], dt.bfloat16, kind="ExternalOutput"
    )

    rg = [list(range(n_cores))]
    xT_r = xT.ap().rearrange("(c p) t -> p c t", p=P)

    with tile.TileContext(nc) as tc:
        with (
            tc.tile_pool(name="const", bufs=1) as cpool,
            tc.tile_pool(name="x", bufs=2) as xpool,
            tc.tile_pool(name="h", bufs=2) as hpool,
            tc.tile_pool(name="o", bufs=2) as opool,
            tc.tile_pool(name="pm", bufs=8) as pmpool,
            tc.tile_pool(name="hps", bufs=2, space="PSUM") as hpsum,
            tc.tile_pool(name="ops", bufs=6, space="PSUM") as opsum,
            tc.tile_pool(name="rsi", bufs=1, space="DRAM") as rspool,
            tc.tile_pool(name="rso", bufs=1, space="DRAM") as rsopool,
        ):
            # group 0's x tile first: 4-way split across DMA queues, with
            # triggers on the (otherwise idle) scalar queue — the sync queue
            # dispatches triggers serially at ~1us each
            ts0 = groups[0][1]
            xt0 = xpool.tile([P, KD, ts0], dt.bfloat16, tag="xt")
            for q in range(3):
                q0 = q * ts0 // 3
                q1 = (q + 1) * ts0 // 3
                nc.scalar.dma_start(xt0[:, :, q0:q1], xT_r[:, :, q0:q1])

            b1_sb = cpool.tile([P, KF], dt.float32)
            nc.scalar.dma_start(b1_sb[:], b1t.ap().rearrange("(c p) -> p c", p=P))

            # ---- FFN weights + remaining constants ----
            # DMA priority classes: the HW queues fair-share HBM bandwidth and
            # a single dma_start only reaches ~50 GB/s on one queue, so each
            # class is issued as several parallel DMAs (aggregate bandwidth),
            # and lower-priority classes are gated behind the critical W1 via
            # sync deps: W1 (waves) -> W2/b2 (parallel) -> remaining x.
            w1_r = w1.ap().rearrange("(c p) f -> p c f", p=P)
            w2_r = w2.ap().rearrange("(c p) n -> p c n", p=P)
            FPW = FF // W1PARTS         # FF columns per W1 part
            JPW = KF // W2PARTS         # f-chunks per W2 part
            w1_parts = []
            w2_parts = []
            prev_class = []  # DMAs of the previous priority wave
            wave_b = []
            for wave, nsplit in (((0,), 4), ((1, 2, 3), 2), ((4, 5, 6, 7), 2)):
                cur = []
                for i in wave:
                    w1p = cpool.tile(
                        [P, KD, FPW], dt.bfloat16, name="w1p", tag=f"w1p{i}"
                    )
                    for q in range(nsplit):
                        f0 = q * FPW // nsplit
                        f1 = (q + 1) * FPW // nsplit
                        d = nc.sync.dma_start(
                            w1p[:, :, f0:f1],
                            w1_r[:, :, i * FPW + f0 : i * FPW + f1],
                        )
                        for pd in prev_class:
                            add_dep_helper(d.ins, pd.ins, True, "w1 wave chain")
                        cur.append(d)
                    w1_parts.append(w1p)
                prev_class = cur
                if i == 0:
                    wave_a = cur
                if i == 3:
                    wave_b = cur
            cur = []
            for i in range(W2PARTS):
                w2p = cpool.tile([P, JPW, D], dt.bfloat16, name="w2p", tag=f"w2p{i}")
                for q in range(2):
                    j0 = q * JPW // 2
                    j1 = (q + 1) * JPW // 2
                    d = nc.sync.dma_start(
                        w2p[:, j0:j1, :], w2_r[:, i * JPW + j0 : i * JPW + j1, :]
                    )
                    # release W2 one W1 wave early so FFN2 g0 isn't starved
                    # (it shares bandwidth with W1's last wave)
                    for pd in wave_b:
                        add_dep_helper(d.ins, pd.ins, True, "w2 after w1 wave b")
                    cur.append(d)
                w2_parts.append(w2p)
            w2_dmas = cur
            b2_b = cpool.tile([P, D], dt.bfloat16)
            d = nc.sync.dma_start(b2_b[:], b2t.ap().partition_broadcast(P))
            for pd in w2_dmas:
                add_dep_helper(d.ins, pd.ins, True, "b2 after weights")

            def w1_ap(kc, j):  # [128 d, 128 f] stationary tile for f-chunk j
                part, jj = divmod(j * P, FPW)
                return w1_parts[part][:, kc, jj : jj + P]

            def w2_ap(j, dsl):  # [128 f, 512 dd] moving tile for f-chunk j
                part, jj = divmod(j, JPW)
                return w2_parts[part][:, jj, dsl]

            # ---- FFN pass over gathered groups ----
            osb_tiles = {}       # sg -> SBUF tile of scaled FFN2 output (bf16)
            pm_tiles = {}        # (si, sg) -> SBUF pm tile
            rs_in = {}           # piece -> DRAM tile
            rs_emitted = [False] * len(pieces)
            nbf0 = len(pieces) - NTAILBF
            rsdt = [dt.float32 if pi < nbf0 else dt.bfloat16
                    for pi in range(len(pieces))]
            off8 = [0] * len(pieces)   # out/outb row offset per piece
            acc_f = acc_b = 0
            for pi, (lo, n) in enumerate(pieces):
                if pi < nbf0:
                    off8[pi] = acc_f
                    acc_f += n * SUB // n_cores
                else:
                    off8[pi] = acc_b
                    acc_b += n * SUB // n_cores

            def emit_ffn1(g, g0, ts):
                if g == 0:
                    xt = xt0
                else:
                    xt = xpool.tile([P, KD, ts], dt.bfloat16, name="xt", tag="xt")
                    d = nc.sync.dma_start(xt[:], xT_r[:, :, g0 : g0 + ts])
                    for wd in (wave_b if g <= 2 else w2_dmas):
                        add_dep_helper(d.ins, wd.ins, True, "x after weights")
                # FFN1: hT[f, t] = gelu(x @ W1 + b1).T in bf16
                ht = hpool.tile([P, KF, ts], dt.bfloat16, name="ht", tag="ht")
                for j in range(KF):
                    hp = hpsum.tile([P, ts], dt.float32, name="hp", tag="hp")
                    for kc in range(KD):
                        nc.tensor.matmul(
                            hp[:], w1_ap(kc, j), xt[:, kc, :],
                            start=(kc == 0), stop=(kc == KD - 1),
                        )
                    nc.scalar.activation(
                        ht[:, j, :], hp[:], GELU_FUNC,
                        bias=b1_sb[:, j : j + 1],
                    )
                return ht

            def emit_ffn2(g0, ts, ht):
                nsub = ts // SUB
                # FFN2 + b2 + routing-weight scale, one sg (128 tokens) at a time
                for s in range(nsub):
                    sg = g0 // SUB + s
                    tsl = slice(s * SUB, (s + 1) * SUB)
                    # prefetch pm pairs consumed right after this sg's FFN2
                    # (~16us of queued PE work hides the DMA latency)
                    for si in si_by_comp[sg]:
                        for b in bands[si]:
                            pmt = pmpool.tile(
                                [SUB, SUB], dt.bfloat16, name="pm", tag="pm"
                            )
                            nc.sync.dma_start(
                                pmt[:], pm_d.ap()[pair_idx[(si, b)], :, :]
                            )
                            pm_tiles[(si, b)] = pmt
                    osb = opool.tile(
                        [P, D], dt.bfloat16, name="osbg", tag="osbg", bufs=osb_live
                    )
                    ops = [
                        opsum.tile([P, 512], dt.float32, name="opsh", tag="opsh")
                        for _ in range(NHALF)
                    ]
                    for j in range(KF):
                        for half in range(NHALF):
                            nc.tensor.matmul(
                                ops[half][:], ht[:, j, tsl],
                                w2_ap(j, slice(half * 512, (half + 1) * 512)),
                                start=(j == 0), stop=(j == KF - 1),
                                skip_group_check=True,
                            )
                    for half in range(NHALF):
                        dsl = slice(half * 512, (half + 1) * 512)
                        nc.vector.tensor_add(osb[:, dsl], ops[half][:], b2_b[:, dsl])
                    osb_tiles[sg] = osb

                    # un-gather every dense si whose band completes at this sg:
                    # dense[si] = sum_b pm[(si,b)].T @ osb[b]
                    for si in si_by_comp[sg]:
                        pi = si_piece[si]
                        if pi not in rs_in:
                            rs_in[pi] = rspool.tile(
                                [pieces[pi][1] * SUB, D], rsdt[pi],
                                name="rsin", tag=f"rsin{pi}",
                            )
                        od = opool.tile([P, D], rsdt[pi], name="od", tag="od")
                        band = bands[si]
                        # final subtiles sit on the serialized tail-collective
                        # chain: run the two half-copies on vector + scalar in
                        # parallel (all gelu is done, so no ACT-table thrash)
                        # and DMA each half as soon as it lands
                        tail_si = comp[si] >= nsg - 2
                        r0 = (si - pieces[pi][0]) * SUB
                        for half in range(NHALF):
                            dsl = slice(half * 512, (half + 1) * 512)
                            pp = opsum.tile(
                                [P, 512], dt.float32, name="opsh", tag="opsh"
                            )
                            for bi, b in enumerate(band):
                                nc.tensor.matmul(
                                    pp[:],
                                    pm_tiles[(si, b)][:],
                                    osb_tiles[b][:, dsl],
                                    start=(bi == 0), stop=(bi == len(band) - 1),
                                    skip_group_check=True,
                                )
                            if tail_si and half == 1:
                                nc.scalar.activation(od[:, dsl], pp[:], Act.Copy)
                            else:
                                nc.vector.tensor_copy(od[:, dsl], pp[:])
                            if tail_si:
                                nc.sync.dma_start(
                                    rs_in[pi][r0 : r0 + SUB, dsl], od[:, dsl]
                                )
                        if not tail_si:
                            nc.sync.dma_start(rs_in[pi][r0 : r0 + SUB, :], od[:])
                        piece_remaining[pi] -= 1

                    # sum expert contributions across cores for finished pieces
                    for pi in range(len(pieces)):
                        if rs_emitted[pi] or piece_remaining[pi] > 0:
                            continue
                        rs_emitted[pi] = True
                        rows = pieces[pi][1] * SUB
                        shard = rows // n_cores
                        rs_out = rsopool.tile(
                            [shard, D], rsdt[pi], name="rso", tag=f"rso{pi}"
                        )
                        nc.gpsimd.collective_compute(
                            "ReduceScatter",
                            mybir.AluOpType.add,
                            replica_groups=rg,
                            ins=[rs_in[pi].opt()],
                            outs=[rs_out.opt()],
                        )
                        dst = out_ext if pi < nbf0 else outb_ext
                        nc.sync.dma_start(
                            dst.ap()[off8[pi] : off8[pi] + shard, :],
                            rs_out[:],
                        )


            # FFN1 leads FFN2 by one group: the startup W2-arrival wait is
            # filled with FFN1(g1) matmuls instead of PE idle
            pend = None
            for g, (g0, ts) in enumerate(groups):
                ht = emit_ffn1(g, g0, ts)
                if pend is not None:
                    emit_ffn2(*pend)
                pend = (g0, ts, ht)
            emit_ffn2(*pend)

    assert all(rs_emitted), "piece schedule failed to emit every collective"
    nc.compile()
    return nc


_NC_CACHE = {}


def _get_nc(cap, bands, pieces, n_cores=NCORES):
    key = (cap, bands, pieces, n_cores)
    if key not in _NC_CACHE:
        _NC_CACHE[key] = _build(*key)
    return _NC_CACHE[key]


def _route(x, wg, bg, wt, bt):
    """Per-token expert selection, same bf16 math as the device gating."""
    def tobf(a):
        return np.asarray(a).astype(BF16).astype(np.float32)

    logits = tobf(x) @ tobf(np.concatenate([wg, wt], axis=1)) + np.concatenate(
        [bg, bt]
    ).astype(np.float32)
    ex = np.exp(logits[:, :E])
    gate = ex / ex.sum(-1, keepdims=True)
    thr = (1.0 / (1.0 + np.exp(-logits[:, E : E + 1]))) * MAX_THRESHOLD
    w = np.maximum(gate - thr, 0.0)
    s = w.sum(-1, keepdims=True)
    w = w / np.where(s == 0, 1.0, s)
    return w > 0, w  # selection mask + routing weights, both [T, E]


def _plan(sel):
    """Gathered-stream plan from the routing table: per-core dense rows (in
    rotated order), capacity, per-si bands (union over cores), RS pieces."""
    order = np.concatenate([np.arange(ROT, T), np.arange(0, ROT)])
    rows = [order[sel[order, k]] for k in range(E)]
    cap = max(SUB, int(-(-max(len(r) for r in rows) // SUB)) * SUB)
    band = {si: set() for si in range(NSI)}
    for k in range(E):
        sg = np.arange(len(rows[k])) // SUB
        si = rows[k] // SUB
        for a, b in set(zip(sg.tolist(), si.tolist())):
            band[b].add(a)
    for si in range(NSI):
        if not band[si]:
            band[si] = {0}
    bands = tuple(tuple(sorted(band[si])) for si in range(NSI))
    # pieces: contiguous si runs over the completion sequence [si0.. , wrap]
    si0 = ROT // SUB
    seq = [(si0 + i) % NSI for i in range(NSI)]
    pieces = []
    pos = 0
    for n in PIECE_SIZES:
        pieces.append((seq[pos], n))
        pos += n
    # emission order = completion order; verify monotonicity is not required
    # (the graph emits each piece when its last si completes)
    return rows, cap, bands, tuple(pieces)


def _make_in_maps(inputs, w1f, b1f, w2f, b2f, we, rows, cap,
                  bands, pieces, n_cores=NCORES):
    """Per-core inputs for the gathered layout."""
    # pm pair emission order must match _build
    comp = [max(b) for b in bands]
    nsg = cap // SUB
    si_by_comp = [[] for _ in range(nsg)]
    for si in range(NSI):
        si_by_comp[comp[si]].append(si)
    pair_idx = {}
    for sg in range(nsg):
        for si in si_by_comp[sg]:
            for b in bands[si]:
                pair_idx[(si, b)] = len(pair_idx)

    maps = []
    for k in range(n_cores):
        rk = rows[k]
        g = np.zeros((cap, D), np.float32)
        g[: len(rk)] = inputs[rk]
        # pm carries the routing weight (not 1.0) so the banded un-gather
        # matmul applies the weighted combine for free
        pm = np.zeros((len(pair_idx), SUB, SUB), np.float32)
        sg_of = np.arange(len(rk)) // SUB
        si_of = rk // SUB
        for i in range(len(rk)):
            key = (int(si_of[i]), int(sg_of[i]))
            if key in pair_idx:
                pm[pair_idx[key], i % SUB, rk[i] % SUB] = we[rk[i], k]
        maps.append({
            "w1": w1f[k].astype(BF16),
            "w2": w2f[k].astype(BF16),
            "b1t": b1f[k].astype(np.float32),
            "b2t": b2f[k].astype(BF16),
            "xT": np.ascontiguousarray(g.T).astype(BF16),
            "pm": pm.astype(BF16),
        })
    return maps


def kernel(inputs, Wg, bg, Wt, bt, W1, b1, W2, b2, _trace=False):
    x = np.asarray(inputs, dtype=np.float32).reshape(-1, D)
    sel, we = _route(x, np.asarray(Wg), np.asarray(bg), np.asarray(Wt),
                     np.asarray(bt))
    rows, cap, bands, pieces = _plan(sel)
    in_maps = _make_in_maps(
        x,
        np.asarray(W1), np.asarray(b1), np.asarray(W2), np.asarray(b2),
        we, rows, cap, bands, pieces,
    )
    nc = _get_nc(cap, bands, pieces)
    res = run_bass_kernel_spmd(
        nc, in_maps, core_ids=list(range(NCORES)), trace=_trace,
    )
    out = _assemble(res.results, pieces, n_cores=NCORES)
    kernel._last_results = res
    return out.reshape(B, S, D)


def _assemble(results, pieces, n_cores):
    """Invert the piece-wise ReduceScatter sharding (f32 + bf16 tail)."""
    nbf0 = len(pieces) - NTAILBF
    out = np.empty((T, D), np.float32)
    for k in range(n_cores):
        rf = np.asarray(results[k]["out"]).astype(np.float32).reshape(-1, D)
        rb = np.asarray(results[k]["outb"]).astype(np.float32).reshape(-1, D)
        off_f = off_b = 0
        for pi, (lo, n) in enumerate(pieces):
            rows = n * SUB
            shard = rows // n_cores
            d0 = lo * SUB + k * shard
            if pi < nbf0:
                out[d0 : d0 + shard] = rf[off_f : off_f + shard]
                off_f += shard
            else:
                out[d0 : d0 + shard] = rb[off_b : off_b + shard]
                off_b += shard
    return out


# revision 39
# speedup vs baseline: 1.0010x; 1.0010x over previous
"""AdaMoE layer on 8 Trainium2 NeuronCores — expert-parallel Bass/Tile kernel.

Strategy: each core k owns expert k. The host routes tokens with the same
bf16 gating math as the reference and builds ONE GLOBAL gathered token
stream per core (only the tokens this core's expert selects, ~65% of 4096),
padded to a capacity computed from the actual data (max count over cores,
rounded up to 128). Routing weights are folded into per-(sg, si) one-hot
permutation blocks (pm), so no gating runs on device. The device runs the
dense FFN in bf16 (fp32 PSUM accumulation) over the gathered stream in
groups of 384 tokens; each 128-token output tile is scattered back to dense
rows via banded permutation matmuls (band pairs computed from the data,
unioned across cores so the SPMD graph is identical on every core, which
the collectives require); dense 128-row tiles are combined across cores by
piece-wise ReduceScatter (512-row pieces, tapered to 256-row pieces at the
end so only one small collective trails the last matmul); the host
reassembles the shards.
"""

import numpy as np
import ml_dtypes

import concourse.bass as bass
import concourse.bacc as bacc
import concourse.mybir as mybir
import concourse.tile as tile
from concourse.tile_rust import add_dep_helper
from concourse.bass_utils import run_bass_kernel_spmd

BF16 = ml_dtypes.bfloat16

B, S, D, FF, E = 2, 2048, 1024, 4096, 8
T = B * S
NCORES = 8
MAX_THRESHOLD = 0.125

P = 128            # SBUF partitions
SUB = 128          # tokens per PE output subtile
KD = D // P        # 8 contraction chunks over D
KF = FF // P       # 32 contraction chunks over FF
NHALF = D // 512   # FFN2 output split (PSUM bank = 512 fp32)
W1PARTS = 8        # W1 DMA split (chained; early f-chunks land earliest)
W2PARTS = 4        # W2 DMA split
GRP = 384          # gathered tokens per FFN1 group (ht SBUF tile)
ROT = 0            # dense-row rotation of the gathered stream
NSI = T // SUB     # dense output subtiles (32)
# RS piece sizes in dense subtiles, in completion order (sum must be NSI).
# Tapered at the end so only small collectives are tail-exposed.
PIECE_SIZES = (4, 4, 4, 4, 4, 4, 2, 1, 1, 2, 2)
NTAILBF = 4        # trailing pieces whose ReduceScatter runs in bf16
                   # (small collectives are latency-bound; bf16 shaves ~3us
                   # each, and only these sit on the serialized tail chain)

dt = mybir.dt
Act = mybir.ActivationFunctionType
GELU_FUNC = Act.Gelu_apprx_tanh


def _build(cap, bands, pieces, n_cores=NCORES):
    """Build the SPMD graph (identical on every core).

    cap: gathered-stream length (multiple of SUB).
    bands: tuple over dense si of tuple-of-sg (ascending) covering it.
    pieces: tuple of (si_lo, n_si) in emission order.
    """
    nsg = cap // SUB
    comp = [max(b) for b in bands]           # completing sg per dense si
    # pm pair index in emission order: si's grouped by completion sg
    si_by_comp = [[] for _ in range(nsg)]
    for si in range(NSI):
        si_by_comp[comp[si]].append(si)
    pair_idx = {}
    for sg in range(nsg):
        for si in si_by_comp[sg]:
            for b in bands[si]:
                pair_idx[(si, b)] = len(pair_idx)
    n_pairs = len(pair_idx)
    # osb liveness window (in sg) for the banded un-gather
    osb_live = max(3, max(comp[si] - min(bands[si]) for si in range(NSI)) + 1)
    # groups of the gathered stream
    groups = []
    g0 = 0
    while g0 < cap:
        ts = min(GRP, cap - g0)
        groups.append((g0, ts))
        g0 += ts
    # piece bookkeeping: which piece each si belongs to + completion sg of piece
    si_piece = {}
    for pi, (lo, n) in enumerate(pieces):
        for si in range(lo, lo + n):
            si_piece[si] = pi
    piece_remaining = [n for (lo, n) in pieces]

    nc = bacc.Bacc(
        "TRN2",
        target_bir_lowering=False,
        debug=False,
        enable_asserts=True,
        num_devices=n_cores,
    )

    xT = nc.dram_tensor("xT", [D, cap], dt.bfloat16, kind="ExternalInput")
    pm_d = nc.dram_tensor("pm", [n_pairs, SUB, SUB], dt.bfloat16,
                          kind="ExternalInput")
    w1 = nc.dram_tensor("w1", [D, FF], dt.bfloat16, kind="ExternalInput")
    w2 = nc.dram_tensor("w2", [FF, D], dt.bfloat16, kind="ExternalInput")
    b1t = nc.dram_tensor("b1t", [FF], dt.float32, kind="ExternalInput")
    b2t = nc.dram_tensor("b2t", [D], dt.bfloat16, kind="ExternalInput")
    n_bf = sum(n for (_, n) in pieces[len(pieces) - NTAILBF :])
    n_f32 = NSI - n_bf
    out_ext = nc.dram_tensor(
        "out", [n_f32 * SUB // n_cores, D], dt.float32, kind="ExternalOutput"
    )
    outb_ext = nc.dram_tensor(
        "outb", [n_bf * SUB // n_cores, D], dt.bfloat16, kind="ExternalOutput"
    )

    rg = [list(range(n_cores))]
    xT_r = xT.ap().rearrange("(c p) t -> p c t", p=P)

    with tile.TileContext(nc) as tc:
        with (
            tc.tile_pool(name="const", bufs=1) as cpool,
            tc.tile_pool(name="x", bufs=2) as xpool,
            tc.tile_pool(name="h", bufs=2) as hpool,
            tc.tile_pool(name="o", bufs=2) as opool,
            tc.tile_pool(name="pm", bufs=8) as pmpool,
            tc.tile_pool(name="hps", bufs=2, space="PSUM") as hpsum,
            tc.tile_pool(name="ops", bufs=6, space="PSUM") as opsum,
            tc.tile_pool(name="rsi", bufs=1, space="DRAM") as rspool,
            tc.tile_pool(name="rso", bufs=1, space="DRAM") as rsopool,
        ):
            # group 0's x tile first: 4-way split across DMA queues, with
            # triggers on the (otherwise idle) scalar queue — the sync queue
            # dispatches triggers serially at ~1us each
            ts0 = groups[0][1]
            xt0 = xpool.tile([P, KD, ts0], dt.bfloat16, tag="xt")
            for q in range(3):
                q0 = q * ts0 // 3
                q1 = (q + 1) * ts0 // 3
                nc.scalar.dma_start(xt0[:, :, q0:q1], xT_r[:, :, q0:q1])

            b1_sb = cpool.tile([P, KF], dt.float32)
            nc.scalar.dma_start(b1_sb[:], b1t.ap().rearrange("(c p) -> p c", p=P))

            # ---- FFN weights + remaining constants ----
            # DMA priority classes: the HW queues fair-share HBM bandwidth and
            # a single dma_start only reaches ~50 GB/s on one queue, so each
            # class is issued as several parallel DMAs (aggregate bandwidth),
            # and lower-priority classes are gated behind the critical W1 via
            # sync deps: W1 (waves) -> W2/b2 (parallel) -> remaining x.
            w1_r = w1.ap().rearrange("(c p) f -> p c f", p=P)
            w2_r = w2.ap().rearrange("(c p) n -> p c n", p=P)
            FPW = FF // W1PARTS         # FF columns per W1 part
            JPW = KF // W2PARTS         # f-chunks per W2 part
            w1_parts = []
            w2_parts = []
            prev_class = []  # DMAs of the previous priority wave
            wave_b = []
            for wave, nsplit in (((0,), 4), ((1, 2, 3), 2), ((4, 5, 6, 7), 2)):
                cur = []
                for i in wave:
                    w1p = cpool.tile(
                        [P, KD, FPW], dt.bfloat16, name="w1p", tag=f"w1p{i}"
                    )
                    for q in range(nsplit):
                        f0 = q * FPW // nsplit
                        f1 = (q + 1) * FPW // nsplit
                        d = nc.sync.dma_start(
                            w1p[:, :, f0:f1],
                            w1_r[:, :, i * FPW + f0 : i * FPW + f1],
                        )
                        for pd in prev_class:
                            add_dep_helper(d.ins, pd.ins, True, "w1 wave chain")
                        cur.append(d)
                    w1_parts.append(w1p)
                prev_class = cur
                if i == 0:
                    wave_a = cur
                if i == 3:
                    wave_b = cur
            cur = []
            for i in range(W2PARTS):
                w2p = cpool.tile([P, JPW, D], dt.bfloat16, name="w2p", tag=f"w2p{i}")
                for q in range(2):
                    j0 = q * JPW // 2
                    j1 = (q + 1) * JPW // 2
                    d = nc.sync.dma_start(
                        w2p[:, j0:j1, :], w2_r[:, i * JPW + j0 : i * JPW + j1, :]
                    )
                    # release W2 one W1 wave early so FFN2 g0 isn't starved
                    # (it shares bandwidth with W1's last wave)
                    for pd in wave_b:
                        add_dep_helper(d.ins, pd.ins, True, "w2 after w1 wave b")
                    cur.append(d)
                w2_parts.append(w2p)
            w2_dmas = cur
            b2_b = cpool.tile([P, D], dt.bfloat16)
            d = nc.sync.dma_start(b2_b[:], b2t.ap().partition_broadcast(P))
            for pd in w2_dmas:
                add_dep_helper(d.ins, pd.ins, True, "b2 after weights")

            def w1_ap(kc, j):  # [128 d, 128 f] stationary tile for f-chunk j
                part, jj = divmod(j * P, FPW)
                return w1_parts[part][:, kc, jj : jj + P]

            def w2_ap(j, dsl):  # [128 f, 512 dd] moving tile for f-chunk j
                part, jj = divmod(j, JPW)
                return w2_parts[part][:, jj, dsl]

            # ---- FFN pass over gathered groups ----
            osb_tiles = {}       # sg -> SBUF tile of scaled FFN2 output (bf16)
            pm_tiles = {}        # (si, sg) -> SBUF pm tile
            rs_in = {}           # piece -> DRAM tile
            rs_emitted = [False] * len(pieces)
            nbf0 = len(pieces) - NTAILBF
            rsdt = [dt.float32 if pi < nbf0 else dt.bfloat16
                    for pi in range(len(pieces))]
            off8 = [0] * len(pieces)   # out/outb row offset per piece
            acc_f = acc_b = 0
            for pi, (lo, n) in enumerate(pieces):
                if pi < nbf0:
                    off8[pi] = acc_f
                    acc_f += n * SUB // n_cores
                else:
                    off8[pi] = acc_b
                    acc_b += n * SUB // n_cores

            def emit_ffn1(g, g0, ts):
                if g == 0:
                    xt = xt0
                else:
                    xt = xpool.tile([P, KD, ts], dt.bfloat16, name="xt", tag="xt")
                    d = nc.sync.dma_start(xt[:], xT_r[:, :, g0 : g0 + ts])
                    for wd in (wave_b if g <= 2 else w2_dmas):
                        add_dep_helper(d.ins, wd.ins, True, "x after weights")
                # FFN1: hT[f, t] = gelu(x @ W1 + b1).T in bf16
                ht = hpool.tile([P, KF, ts], dt.bfloat16, name="ht", tag="ht")
                for j in range(KF):
                    hp = hpsum.tile([P, ts], dt.float32, name="hp", tag="hp")
                    for kc in range(KD):
                        nc.tensor.matmul(
                            hp[:], w1_ap(kc, j), xt[:, kc, :],
                            start=(kc == 0), stop=(kc == KD - 1),
                        )
                    nc.scalar.activation(
                        ht[:, j, :], hp[:], GELU_FUNC,
                        bias=b1_sb[:, j : j + 1],
                    )
                return ht

            def emit_ffn2(g0, ts, ht):
                nsub = ts // SUB
                # FFN2 + b2 + routing-weight scale, one sg (128 tokens) at a time
                for s in range(nsub):
                    sg = g0 // SUB + s
                    tsl = slice(s * SUB, (s + 1) * SUB)
                    # prefetch pm pairs consumed right after this sg's FFN2
                    # (~16us of queued PE work hides the DMA latency)
                    for si in si_by_comp[sg]:
                        for b in bands[si]:
                            pmt = pmpool.tile(
                                [SUB, SUB], dt.bfloat16, name="pm", tag="pm"
                            )
                            nc.sync.dma_start(
                                pmt[:], pm_d.ap()[pair_idx[(si, b)], :, :]
                            )
                            pm_tiles[(si, b)] = pmt
                    osb = opool.tile(
                        [P, D], dt.bfloat16, name="osbg", tag="osbg", bufs=osb_live
                    )
                    ops = [
                        opsum.tile([P, 512], dt.float32, name="opsh", tag="opsh")
                        for _ in range(NHALF)
                    ]
                    for j in range(KF):
                        for half in range(NHALF):
                            nc.tensor.matmul(
                                ops[half][:], ht[:, j, tsl],
                                w2_ap(j, slice(half * 512, (half + 1) * 512)),
                                start=(j == 0), stop=(j == KF - 1),
                                skip_group_check=True,
                            )
                    for half in range(NHALF):
                        dsl = slice(half * 512, (half + 1) * 512)
                        nc.vector.tensor_add(osb[:, dsl], ops[half][:], b2_b[:, dsl])
                    osb_tiles[sg] = osb

                    # un-gather every dense si whose band completes at this sg:
                    # dense[si] = sum_b pm[(si,b)].T @ osb[b]
                    for si in si_by_comp[sg]:
                        pi = si_piece[si]
                        if pi not in rs_in:
                            rs_in[pi] = rspool.tile(
                                [pieces[pi][1] * SUB, D], rsdt[pi],
                                name="rsin", tag=f"rsin{pi}",
                            )
                        od = opool.tile([P, D], rsdt[pi], name="od", tag="od")
                        band = bands[si]
                        # final subtiles sit on the serialized tail-collective
                        # chain: run the two half-copies on vector + scalar in
                        # parallel (all gelu is done, so no ACT-table thrash)
                        # and DMA each half as soon as it lands
                        tail_si = comp[si] >= nsg - 2
                        r0 = (si - pieces[pi][0]) * SUB
                        for half in range(NHALF):
                            dsl = slice(half * 512, (half + 1) * 512)
                            pp = opsum.tile(
                                [P, 512], dt.float32, name="opsh", tag="opsh"
                            )
                            for bi, b in enumerate(band):
                                nc.tensor.matmul(
                                    pp[:],
                                    pm_tiles[(si, b)][:],
                                    osb_tiles[b][:, dsl],
                                    start=(bi == 0), stop=(bi == len(band) - 1),
                                    skip_group_check=True,
                                )
                            if tail_si and half == 1:
                                nc.scalar.activation(od[:, dsl], pp[:], Act.Copy)
                            else:
                                nc.vector.tensor_copy(od[:, dsl], pp[:])
                            if tail_si:
                                nc.sync.dma_start(
                                    rs_in[pi][r0 : r0 + SUB, dsl], od[:, dsl]
                                )
                        if not tail_si:
                            nc.sync.dma_start(rs_in[pi][r0 : r0 + SUB, :], od[:])
                        piece_remaining[pi] -= 1

                    # sum expert contributions across cores for finished pieces
                    for pi in range(len(pieces)):
                        if rs_emitted[pi] or piece_remaining[pi] > 0:
                            continue
                        rs_emitted[pi] = True
                        rows = pieces[pi][1] * SUB
                        shard = rows // n_cores
                        rs_out = rsopool.tile(
                            [shard, D], rsdt[pi], name="rso", tag=f"rso{pi}"
                        )
                        nc.gpsimd.collective_compute(
                            "ReduceScatter",
                            mybir.AluOpType.add,
                            replica_groups=rg,
                            ins=[rs_in[pi].opt()],
                            outs=[rs_out.opt()],
                        )
                        dst = out_ext if pi < nbf0 else outb_ext
                        nc.sync.dma_start(
                            dst.ap()[off8[pi] : off8[pi] + shard, :],
                            rs_out[:],
                        )


            # FFN1 leads FFN2 by one group: the startup W2-arrival wait is
            # filled with FFN1(g1) matmuls instead of PE idle
            pend = None
            for g, (g0, ts) in enumerate(groups):
                ht = emit_ffn1(g, g0, ts)
                if pend is not None:
                    emit_ffn2(*pend)
                pend = (g0, ts, ht)
            emit_ffn2(*pend)

    assert all(rs_emitted), "piece schedule failed to emit every collective"
    nc.compile()
    return nc


_NC_CACHE = {}


def _get_nc(cap, bands, pieces, n_cores=NCORES):
    key = (cap, bands, pieces, n_cores)
    if key not in _NC_CACHE:
        _NC_CACHE[key] = _build(*key)
    return _NC_CACHE[key]


def _route(x, wg, bg, wt, bt):
    """Per-token expert selection, same bf16 math as the device gating."""
    def tobf(a):
        return np.asarray(a).astype(BF16).astype(np.float32)

    logits = tobf(x) @ tobf(np.concatenate([wg, wt], axis=1)) + np.concatenate(
        [bg, bt]
    ).astype(np.float32)
    ex = np.exp(logits[:, :E])
    gate = ex / ex.sum(-1, keepdims=True)
    thr = (1.0 / (1.0 + np.exp(-logits[:, E : E + 1]))) * MAX_THRESHOLD
    w = np.maximum(gate - thr, 0.0)
    s = w.sum(-1, keepdims=True)
    w = w / np.where(s == 0, 1.0, s)
    return w > 0, w  # selection mask + routing weights, both [T, E]


def _plan(sel):
    """Gathered-stream plan from the routing table: per-core dense rows (in
    rotated order), capacity, per-si bands (union over cores), RS pieces."""
    order = np.concatenate([np.arange(ROT, T), np.arange(0, ROT)])
    rows = [order[sel[order, k]] for k in range(E)]
    cap = max(SUB, int(-(-max(len(r) for r in rows) // SUB)) * SUB)
    band = {si: set() for si in range(NSI)}
    for k in range(E):
        sg = np.arange(len(rows[k])) // SUB
        si = rows[k] // SUB
        for a, b in set(zip(sg.tolist(), si.tolist())):
            band[b].add(a)
    for si in range(NSI):
        if not band[si]:
            band[si] = {0}
    bands = tuple(tuple(sorted(band[si])) for si in range(NSI))
    # pieces: contiguous si runs over the completion sequence [si0.. , wrap]
    si0 = ROT // SUB
    seq = [(si0 + i) % NSI for i in range(NSI)]
    pieces = []
    pos = 0
    for n in PIECE_SIZES:
        pieces.append((seq[pos], n))
        pos += n
    # emission order = completion order; verify monotonicity is not required
    # (the graph emits each piece when its last si completes)
    return rows, cap, bands, tuple(pieces)


def _make_in_maps(inputs, w1f, b1f, w2f, b2f, we, rows, cap,
                  bands, pieces, n_cores=NCORES):
    """Per-core inputs for the gathered layout."""
    # pm pair emission order must match _build
    comp = [max(b) for b in bands]
    nsg = cap // SUB
    si_by_comp = [[] for _ in range(nsg)]
    for si in range(NSI):
        si_by_comp[comp[si]].append(si)
    pair_idx = {}
    for sg in range(nsg):
        for si in si_by_comp[sg]:
            for b in bands[si]:
                pair_idx[(si, b)] = len(pair_idx)

    maps = []
    for k in range(n_cores):
        rk = rows[k]
        g = np.zeros((cap, D), np.float32)
        g[: len(rk)] = inputs[rk]
        # pm carries the routing weight (not 1.0) so the banded un-gather
        # matmul applies the weighted combine for free
        pm = np.zeros((len(pair_idx), SUB, SUB), np.float32)
        sg_of = np.arange(len(rk)) // SUB
        si_of = rk // SUB
        for i in range(len(rk)):
            key = (int(si_of[i]), int(sg_of[i]))
            if key in pair_idx:
                pm[pair_idx[key], i % SUB, rk[i] % SUB] = we[rk[i], k]
        maps.append({
            "w1": w1f[k].astype(BF16),
            "w2": w2f[k].astype(BF16),
            "b1t": b1f[k].astype(np.float32),
            "b2t": b2f[k].astype(BF16),
            "xT": np.ascontiguousarray(g.T).astype(BF16),
            "pm": pm.astype(BF16),
        })
    return maps


def kernel(inputs, Wg, bg, Wt, bt, W1, b1, W2, b2, _trace=False):
    x = np.asarray(inputs, dtype=np.float32).reshape(-1, D)
    sel, we = _route(x, np.asarray(Wg), np.asarray(bg), np.asarray(Wt),
                     np.asarray(bt))
    rows, cap, bands, pieces = _plan(sel)
    in_maps = _make_in_maps(
        x,
        np.asarray(W1), np.asarray(b1), np.asarray(W2), np.asarray(b2),
        we, rows, cap, bands, pieces,
    )
    nc = _get_nc(cap, bands, pieces)
    res = run_bass_kernel_spmd(
        nc, in_maps, core_ids=list(range(NCORES)), trace=_trace,
    )
    out = _assemble(res.results, pieces, n_cores=NCORES)
    kernel._last_results = res
    return out.reshape(B, S, D)


def _assemble(results, pieces, n_cores):
    """Invert the piece-wise ReduceScatter sharding (f32 + bf16 tail)."""
    nbf0 = len(pieces) - NTAILBF
    out = np.empty((T, D), np.float32)
    for k in range(n_cores):
        rf = np.asarray(results[k]["out"]).astype(np.float32).reshape(-1, D)
        rb = np.asarray(results[k]["outb"]).astype(np.float32).reshape(-1, D)
        off_f = off_b = 0
        for pi, (lo, n) in enumerate(pieces):
            rows = n * SUB
            shard = rows // n_cores
            d0 = lo * SUB + k * shard
            if pi < nbf0:
                out[d0 : d0 + shard] = rf[off_f : off_f + shard]
                off_f += shard
            else:
                out[d0 : d0 + shard] = rb[off_b : off_b + shard]
                off_b += shard
    return out
